# revision 14
# baseline (speedup 1.0000x reference)
"""Trainium2 Bass kernel for nn_ALNNLayer (ALNN attention-like layer).

Reference computation (per batch b, ref-time k, step l, feature d):
    dist  = |T[b,l,d] - r_k|                      r_k = linspace(0,48,13)
    kern  = exp(-relu(alpha_k) * dist)
    inten = relu(X * kern) = relu(X) * kern       (kern > 0)
    pre   = wt0*X + wt1*DT + wt2*inten + wt3*M + 4*bt
    lat   = relu(pre)
    out[b,k,d] = relu( sum_l wv*lat + 200*bv[k,d] )

Strategy: data-parallel over batch (8 cores x 8 batches). Per core the
SBUF layout is [100 l-partitions, (j=l//100, b, d) free]; weights are
broadcast over b with stride-0 access patterns. Engine split:
  - VectorE: packed bf16 products (X*wt0 | DT*wt1 || M*wt3 | relu(X)*wt2)
    as two [100, 2048] ops, kern-apply (nonzero alpha_k only), wv multiply,
    final bias+relu epilogue
  - ScalarE: |T-r_k| and exp (front-loaded; only needs T+S), and the
    relu fused into the PSUM eviction
  - TensorE: term summation via identity matmuls accumulating in PSUM
    (bias first via a b-broadcast AP so accumulation starts before the
    products land), and the L-reduction via a k-column selector matmul
Schedule details: host pre-transposes all inputs so every DMA is
contiguous >=1KB-per-partition chunks; DMAs spread over the sync,
scalar, gpsimd and tensor HWDGE queues; all 13 W tiles are resident in
SBUF; dummy matmuls + a dummy activation during the DMA phase warm the
PE HAM clock gate and hoist the ACT table load off the critical path;
zero-alpha and nonzero-alpha k's are interleaved so ACT's dist/exp work
spreads evenly between PSUM evictions.
k's with relu(alpha_k) == 0 skip dist/exp/kern entirely (kern == 1);
the NEFF is compiled per alpha-sign-pattern, so this stays correct for
any inputs.
"""

import sys

for _p in ("/opt/trn_rl_repo", "/root/.axon_site/_ro/trn_rl_repo"):
    if _p not in sys.path:
        sys.path.append(_p)

import numpy as np
import ml_dtypes

import concourse.bass as bass
import concourse.bacc as bacc
import concourse.tile as tile
from concourse import mybir
from concourse.bass_utils import run_bass_kernel_spmd

B, L, D, K = 64, 200, 64, 13
NCORES = 8
BLOC = B // NCORES  # 8
PRIOR_HOURS = 48.0
REF_TIME = np.linspace(0.0, PRIOR_HOURS, K).astype(np.float32)

LP = 100            # l partitions
LJ = 2              # l super-tiles (l = j*LP + p)
NF = 4              # packed product features: X, DT, M, relu(X)
FREE = LJ * BLOC * D  # 1024

F32 = mybir.dt.float32
BF16 = mybir.dt.bfloat16
AX = mybir.AluOpType
AF = mybir.ActivationFunctionType
NPBF = ml_dtypes.bfloat16

N_WARM_MM = 40      # dummy matmuls to warm the PE HAM clock gate
WGROUP = 4          # k's per W-DMA group (k's packed in consumption order)


def k_order(nonzero):
    """Zero-alpha k's interleaved with nonzero so ACT work spreads out."""
    zs = [k for k in range(K) if not nonzero[k]]
    nzs = [k for k in range(K) if nonzero[k]]
    order = []
    while zs or nzs:
        if zs:
            order.append(zs.pop(0))
        if nzs:
            order.append(nzs.pop(0))
    return order


def _bc(ap, nb=BLOC):
    """Insert a stride-0 b dim before the last free dim of an AP."""
    return bass.AP(
        tensor=ap.tensor, offset=ap.offset,
        ap=list(ap.ap[:-1]) + [[0, nb], ap.ap[-1]],
    )


def build_bass(nonzero):
    """nonzero: tuple of bool per k — whether relu(alpha_k) > 0."""
    nc = bacc.Bacc("TRN2", target_bir_lowering=False, debug=False)

    # host-pretransposed inputs: [j, p, b, d] with l = j*LP + p
    T_d = nc.declare_dram_parameter("T4", [LJ, LP, BLOC, D], F32, isOutput=False)
    X_d = nc.declare_dram_parameter("X4", [LJ, LP, BLOC, D], BF16, isOutput=False)
    DT_d = nc.declare_dram_parameter("DT4", [LJ, LP, BLOC, D], BF16, isOutput=False)
    M_d = nc.declare_dram_parameter("M4", [LJ, LP, BLOC, D], BF16, isOutput=False)
    # per-k weights in CONSUMPTION order (host pre-permuted by k_order):
    # [K, LP, 6, LJ, D]: products (wt0, wt1, wt3, wt2) | 4bt | wv
    W_d = nc.declare_dram_parameter("W", [K, LP, NF + 2, LJ, D], BF16, isOutput=False)
    S_d = nc.declare_dram_parameter("S", [128, 2 * K], F32, isOutput=False)
    BV_d = nc.declare_dram_parameter("BV", [K, D], F32, isOutput=False)  # 200*b_v
    E_d = nc.declare_dram_parameter("ESEL", [128, K * K], BF16, isOutput=False)
    out_d = nc.declare_dram_parameter("out", [BLOC, K, D], F32, isOutput=True)

    order = k_order(nonzero)
    groups = [list(range(g, min(g + WGROUP, K))) for g in range(0, K, WGROUP)]

    from contextlib import ExitStack

    with tile.TileContext(nc) as tc, ExitStack() as ctx:
        const = ctx.enter_context(tc.tile_pool(name="const", bufs=1))
        tmp = ctx.enter_context(tc.tile_pool(name="tmp", bufs=3))
        psum = ctx.enter_context(tc.tile_pool(name="psum", bufs=3, space="PSUM"))
        psum1 = ctx.enter_context(tc.tile_pool(name="psum1", bufs=1, space="PSUM"))
        psumw = ctx.enter_context(tc.tile_pool(name="psumw", bufs=1, space="PSUM"))

        # ---- DMAs: spread across queues; every transfer is contiguous ----
        # scalar queue: the three packed-product inputs, needed first
        Dp = const.tile([LP, NF, LJ, BLOC, D], BF16, tag="Dp")
        for f, dram in ((0, X_d), (1, DT_d), (2, M_d)):
            nc.scalar.dma_start(
                out=Dp[:, f], in_=dram[:].rearrange("j p b d -> p j b d")
            )

        # sync queue: S, first/third W group, eye, selector, BV
        S_sb = const.tile([128, 2 * K], F32)
        nc.sync.dma_start(out=S_sb[:], in_=S_d[:])
        EYE_d = nc.declare_dram_parameter("EYE", [LP, LP], BF16, isOutput=False)
        EYE = const.tile([LP, LP], BF16, tag="eye")
        Wg = []
        for gi, g in enumerate(groups):
            wg_tile = const.tile(
                [LP, len(g), NF + 2, LJ, D], BF16, tag=f"Wg{gi}", name=f"Wg{gi}"
            )
            Wg.append(wg_tile)

        def wslot(i):
            return Wg[i // WGROUP][:, i % WGROUP]

        nc.sync.dma_start(out=Wg[0][:], in_=W_d[groups[0][0] : groups[0][-1] + 1]
                          .rearrange("k p f j d -> p k f j d"))
        nc.sync.dma_start(out=EYE[:], in_=EYE_d[:])
        E_sb = const.tile([128, K * K], BF16)
        nc.sync.dma_start(out=E_sb[:], in_=E_d[:])
        if len(groups) > 2:
            nc.sync.dma_start(out=Wg[2][:], in_=W_d[groups[2][0] : groups[2][-1] + 1]
                              .rearrange("k p f j d -> p k f j d"))
        BV_sb = const.tile([K, D], F32)
        nc.sync.dma_start(out=BV_sb[:], in_=BV_d[:])

        # gpsimd queue: T, second/fourth W group
        Tt = const.tile([LP, LJ, BLOC, D], F32, tag="T")
        nc.gpsimd.dma_start(out=Tt[:], in_=T_d[:].rearrange("j p b d -> p j b d"))
        for gi in (1, 3):
            if gi < len(groups):
                nc.gpsimd.dma_start(
                    out=Wg[gi][:], in_=W_d[groups[gi][0] : groups[gi][-1] + 1]
                    .rearrange("k p f j d -> p k f j d"))

        # ---- PE warm-up: memset a zero tile, then dummy matmuls ----
        warm = const.tile([128, 192], BF16, tag="warm")
        nc.vector.memset(warm[:], 0.0)
        pw = psumw.tile([128, 64], F32, tag="pw")
        for _ in range(N_WARM_MM):
            nc.tensor.matmul(pw[:], warm[:, :128], warm[:, 128:], start=True, stop=True)
        # ACT table-load hoist (Exp set also holds Abs/Relu/Copy)
        dummy_act = const.tile([1, 8], F32, tag="dact")
        nc.scalar.activation(dummy_act[:], warm[:1, :8], AF.Exp)

        # f3 slot <- relu(X)
        nc.vector.tensor_scalar_max(Dp[:, 3], Dp[:, 0], 0.0)

        # ---- front-loaded dist/exp (needs only T + S) ----
        kerns = {}

        def emit_distexp(k):
            dist = tmp.tile([LP, LJ, BLOC, D], F32, tag="dist")
            nc.scalar.activation(
                dist[:], Tt[:], AF.Abs,
                bias=S_sb[:LP, K + k : K + k + 1], scale=1.0,
            )
            kern = const.tile([LP, LJ, BLOC, D], BF16, tag=f"kern{k}")
            nc.scalar.activation(kern[:], dist[:], AF.Exp, scale=S_sb[:LP, k : k + 1])
            kerns[k] = kern

        osb = const.tile([K, BLOC, D], F32)
        po = psum1.tile([K, BLOC, D], F32)  # L-sums, one bank, rows = k

        Sps, Qs = {}, {}

        def stage_product(i):
            k = order[i]
            if nonzero[k]:
                emit_distexp(k)
            w = wslot(i)
            Sp = tmp.tile([LP, NF, LJ, BLOC, D], BF16, tag="Sp")
            for f0 in (0, 2):
                wap = bass.AP(
                    tensor=w.tensor,
                    offset=w.offset + f0 * LJ * D,
                    ap=[w.ap[0], [LJ * D, 2], [D, LJ], [0, BLOC], [1, D]],
                )
                nc.vector.tensor_tensor(
                    Sp[:, f0 : f0 + 2], Dp[:, f0 : f0 + 2], wap, AX.mult
                )
            Sps[i] = Sp
            if nonzero[k]:
                Q = tmp.tile([LP, LJ, BLOC, D], BF16, tag="Q")
                nc.vector.tensor_tensor(Q[:], Sp[:, 3], kerns[k][:], AX.mult)
                Qs[i] = Q

        def stage_mms(i):
            Sp = Sps[i]
            qterm = Qs.get(i)
            w = wslot(i)
            # bias first: only needs W (starts accumulation before products)
            terms = [_bc(w[:, NF])]
            terms += [Sp[:, 0], Sp[:, 1], Sp[:, 2]]
            terms.append(qterm[:] if qterm is not None else Sp[:, 3])
            pre = psum.tile([LP, LJ, BLOC, D], F32, tag="pre")
            for ti, t in enumerate(terms):
                for j in range(LJ):
                    nc.tensor.matmul(
                        pre[:, j], EYE[:LP, :LP], t[:, j],
                        start=(ti == 0), stop=(ti == len(terms) - 1),
                    )
            return pre

        pres = {}

        def stage_back(i, first, last):
            k = order[i]
            pre = pres[i]
            w = wslot(i)
            lat = tmp.tile([LP, LJ, BLOC, D], BF16, tag="lat")
            nc.scalar.activation(lat[:], pre[:], AF.Relu)
            z = tmp.tile([LP, LJ, BLOC, D], BF16, tag="z")
            nc.vector.tensor_tensor(z[:], lat[:], _bc(w[:, NF + 1]), AX.mult)
            for j in range(LJ):
                nc.tensor.matmul(
                    po[:, :, :],
                    E_sb[:LP, k * K : (k + 1) * K],
                    z[:, j],
                    start=(first and j == 0),
                    stop=(last and j == LJ - 1),
                )

        stage_product(0)
        stage_product(1)
        for i in range(K):
            if i + 2 < K:
                stage_product(i + 2)
            pres[i] = stage_mms(i)
            if i >= 1:
                stage_back(i - 1, first=(i == 1), last=False)
        stage_back(K - 1, first=False, last=True)

        # ---- epilogue: out = relu(po + 200*bv) ----
        nc.vector.tensor_tensor(osb[:], po[:], _bc(BV_sb[:]), AX.add)
        nc.vector.tensor_scalar_max(osb[:], osb[:], 0.0)
        nc.scalar.dma_start(out=out_d[:].rearrange("b k d -> k b d"), in_=osb[:])

    nc.compile()
    return nc


_NC_CACHE = {}


def _get_nc(nonzero):
    key = tuple(nonzero)
    if key not in _NC_CACHE:
        _NC_CACHE[key] = build_bass(key)
    return _NC_CACHE[key]


def make_in_maps(X, T, M, DT, alpha, w_v, w_t, b_v, b_t):
    X = np.asarray(X, np.float32)
    T = np.asarray(T, np.float32)
    M = np.asarray(M, np.float32)
    DT = np.asarray(DT, np.float32)
    w_t = np.asarray(w_t, np.float32)
    w_v = np.asarray(w_v, np.float32)
    b_t = np.asarray(b_t, np.float32)
    b_v = np.asarray(b_v, np.float32)
    alpha = np.asarray(alpha, np.float32).reshape(K)

    nonzero = tuple(bool(a > 0) for a in alpha)
    order = k_order(nonzero)

    # weight pack: [K, L, 6, D] with f-order (wt0, wt1, wt3, wt2, 4bt, wv)
    W = np.empty((K, L, NF + 2, D), np.float32)
    W[:, :, 0] = w_t[:, :, :, 0]
    W[:, :, 1] = w_t[:, :, :, 1]
    W[:, :, 2] = w_t[:, :, :, 3]
    W[:, :, 3] = w_t[:, :, :, 2]
    W[:, :, 4] = 4.0 * b_t[:, :, :, 0]
    W[:, :, 5] = w_v
    # -> [K, LP, 6, LJ, D] with l = j*LP + p, k's in consumption order
    W = W.reshape(K, LJ, LP, NF + 2, D).transpose(0, 2, 3, 1, 4)[list(order)]
    W = np.ascontiguousarray(W).astype(NPBF)

    S = np.tile(
        np.concatenate(
            [-np.maximum(alpha.reshape(1, K), 0.0), -REF_TIME.reshape(1, K)], axis=1
        ),
        (128, 1),
    ).astype(np.float32)
    BV = (float(L) * b_v[:, 0, :]).astype(np.float32)
    ESEL = np.zeros((128, K * K), np.float32)
    for k in range(K):
        ESEL[:, k * K + k] = 1.0
    ESEL = ESEL.astype(NPBF)
    EYE = np.eye(LP, dtype=np.float32).astype(NPBF)

    def tr4(A):
        # [BLOC, L, D] -> [LJ, LP, BLOC, D]
        return np.ascontiguousarray(
            A.reshape(BLOC, LJ, LP, D).transpose(1, 2, 0, 3)
        )

    in_maps = []
    for c in range(NCORES):
        b0 = c * BLOC
        in_maps.append(
            {
                "T4": tr4(T[b0 : b0 + BLOC]),
                "X4": tr4(X[b0 : b0 + BLOC].astype(NPBF)),
                "DT4": tr4(DT[b0 : b0 + BLOC].astype(NPBF)),
                "M4": tr4(M[b0 : b0 + BLOC].astype(NPBF)),
                "W": W,
                "S": S,
                "BV": BV,
                "ESEL": ESEL,
                "EYE": EYE,
            }
        )
    return in_maps, nonzero


def kernel(X, T, M, DT, alpha, w_v, w_t, b_v, b_t):
    in_maps, nonzero = make_in_maps(X, T, M, DT, alpha, w_v, w_t, b_v, b_t)
    nc = _get_nc(nonzero)
    res = run_bass_kernel_spmd(nc, in_maps, core_ids=list(range(NCORES)))
    out = np.concatenate([res.results[c]["out"] for c in range(NCORES)], axis=0)
    return out.astype(np.float32)


# revision 17
# speedup vs baseline: 1.0450x; 1.0450x over previous
"""Trainium2 Bass kernel for nn_ALNNLayer (ALNN attention-like layer).

Reference computation (per batch b, ref-time k, step l, feature d):
    dist  = |T[b,l,d] - r_k|                      r_k = linspace(0,48,13)
    kern  = exp(-relu(alpha_k) * dist)
    inten = relu(X * kern) = relu(X) * kern       (kern > 0)
    pre   = wt0*X + wt1*DT + wt2*inten + wt3*M + 4*bt
    lat   = relu(pre)
    out[b,k,d] = relu( sum_l wv*lat + 200*bv[k,d] )

Strategy: data-parallel over batch (8 cores x 8 batches). Per core the
SBUF layout is [100 l-partitions, (j=l//100, b, d) free]; weights are
broadcast over b with stride-0 access patterns. Engine split:
  - VectorE: packed bf16 products (X*wt0 | DT*wt1 || M*wt3 | relu(X)*wt2)
    as two [100, 2048] ops, kern-apply (nonzero alpha_k only), wv multiply,
    final bias+relu epilogue
  - ScalarE: |T-r_k| and exp (front-loaded; only needs T+S), and the
    relu fused into the PSUM eviction
  - TensorE: term summation via identity matmuls accumulating in PSUM
    (bias first via a b-broadcast AP so accumulation starts before the
    products land), and the L-reduction via a k-column selector matmul
Schedule details: host pre-transposes all inputs so every DMA is
contiguous >=1KB-per-partition chunks; DMAs spread over the sync,
scalar, gpsimd and tensor HWDGE queues; all 13 W tiles are resident in
SBUF; dummy matmuls + a dummy activation during the DMA phase warm the
PE HAM clock gate and hoist the ACT table load off the critical path;
zero-alpha and nonzero-alpha k's are interleaved so ACT's dist/exp work
spreads evenly between PSUM evictions.
k's with relu(alpha_k) == 0 skip dist/exp/kern entirely (kern == 1);
the NEFF is compiled per alpha-sign-pattern, so this stays correct for
any inputs.
"""

import sys

for _p in ("/opt/trn_rl_repo", "/root/.axon_site/_ro/trn_rl_repo"):
    if _p not in sys.path:
        sys.path.append(_p)

import numpy as np
import ml_dtypes

import concourse.bass as bass
import concourse.bacc as bacc
import concourse.tile as tile
from concourse import mybir
from concourse.bass_utils import run_bass_kernel_spmd

B, L, D, K = 64, 200, 64, 13
NCORES = 8
BLOC = B // NCORES  # 8
PRIOR_HOURS = 48.0
REF_TIME = np.linspace(0.0, PRIOR_HOURS, K).astype(np.float32)

LP = 100            # l partitions
LJ = 2              # l super-tiles (l = j*LP + p)
NF = 4              # packed product features: X, DT, M, relu(X)
FREE = LJ * BLOC * D  # 1024

F32 = mybir.dt.float32
BF16 = mybir.dt.bfloat16
AX = mybir.AluOpType
AF = mybir.ActivationFunctionType
NPBF = ml_dtypes.bfloat16

N_WARM_MM = 40      # dummy matmuls to warm the PE HAM clock gate
WGROUP = 4          # k's per W-DMA group (k's packed in consumption order)


def k_order(nonzero):
    """Zero-alpha k's interleaved with nonzero so ACT work spreads out."""
    zs = [k for k in range(K) if not nonzero[k]]
    nzs = [k for k in range(K) if nonzero[k]]
    order = []
    while zs or nzs:
        if zs:
            order.append(zs.pop(0))
        if nzs:
            order.append(nzs.pop(0))
    return order


def _bc(ap, nb=BLOC):
    """Insert a stride-0 b dim before the last free dim of an AP."""
    return bass.AP(
        tensor=ap.tensor, offset=ap.offset,
        ap=list(ap.ap[:-1]) + [[0, nb], ap.ap[-1]],
    )


def build_bass(nonzero):
    """nonzero: tuple of bool per k — whether relu(alpha_k) > 0."""
    nc = bacc.Bacc("TRN2", target_bir_lowering=False, debug=False)

    # host-pretransposed inputs: [j, p, b, d] with l = j*LP + p
    # D4 packs the four product features (X, DT, M, relu(X)) in one blob
    D4_d = nc.declare_dram_parameter("D4", [NF, LJ, LP, BLOC, D], BF16, isOutput=False)
    T_d = nc.declare_dram_parameter("T4", [LJ, LP, BLOC, D], F32, isOutput=False)
    # per-k weights in CONSUMPTION order (host pre-permuted by k_order):
    # [K, LP, 6, LJ, D]: products (wt0, wt1, wt3, wt2) | 4bt | wv
    W_d = nc.declare_dram_parameter("W", [K, LP, NF + 2, LJ, D], BF16, isOutput=False)
    # CF: [128, 26+64] f32 = S (exp scales | abs biases) | 200*b_v padded
    CF_d = nc.declare_dram_parameter("CF", [128, 2 * K + D], F32, isOutput=False)
    # CB: [128, 100+169] bf16 = eye(100) padded | k-column selector blocks
    CB_d = nc.declare_dram_parameter("CB", [128, LP + K * K], BF16, isOutput=False)
    out_d = nc.declare_dram_parameter("out", [BLOC, K, D], F32, isOutput=True)

    order = k_order(nonzero)
    groups = [list(range(g, min(g + WGROUP, K))) for g in range(0, K, WGROUP)]

    from contextlib import ExitStack

    with tile.TileContext(nc) as tc, ExitStack() as ctx:
        const = ctx.enter_context(tc.tile_pool(name="const", bufs=1))
        tmp = ctx.enter_context(tc.tile_pool(name="tmp", bufs=3))
        psum = ctx.enter_context(tc.tile_pool(name="psum", bufs=3, space="PSUM"))
        psum1 = ctx.enter_context(tc.tile_pool(name="psum1", bufs=1, space="PSUM"))
        psumw = ctx.enter_context(tc.tile_pool(name="psumw", bufs=1, space="PSUM"))

        # ---- DMAs: HWDGE only (sync + scalar), ordered by first use ----
        Wg = []
        for gi, g in enumerate(groups):
            wg_tile = const.tile(
                [LP, len(g), NF + 2, LJ, D], BF16, tag=f"Wg{gi}", name=f"Wg{gi}"
            )
            Wg.append(wg_tile)

        def wslot(i):
            return Wg[i // WGROUP][:, i % WGROUP]

        def wg_dma(eng, gi):
            eng.dma_start(
                out=Wg[gi][:],
                in_=W_d[groups[gi][0] : groups[gi][-1] + 1]
                .rearrange("k p f j d -> p k f j d"),
            )

        # sync queue: consts (tiny), W groups 0 and 2
        CF = const.tile([128, 2 * K + D], F32, tag="CF")
        nc.sync.dma_start(out=CF[:], in_=CF_d[:])
        S_sb = CF[:, : 2 * K]
        BV_sb = CF[:K, 2 * K :]
        wg_dma(nc.sync, 0)
        CB = const.tile([128, LP + K * K], BF16, tag="CB")
        nc.sync.dma_start(out=CB[:], in_=CB_d[:])
        EYE = CB[:LP, :LP]
        E_sb = CB[:, LP:]
        if len(groups) > 2:
            wg_dma(nc.sync, 2)

        # scalar queue: packed-product inputs, T, W groups 1 and 3
        Dp = const.tile([LP, NF, LJ, BLOC, D], BF16, tag="Dp")
        nc.scalar.dma_start(
            out=Dp[:], in_=D4_d[:].rearrange("f j p b d -> p f j b d")
        )
        Tt = const.tile([LP, LJ, BLOC, D], F32, tag="T")
        nc.scalar.dma_start(out=Tt[:], in_=T_d[:].rearrange("j p b d -> p j b d"))

        # ---- PE warm-up + ACT table-load hoist during the DMA phase ----
        warm = const.tile([128, 192], BF16, tag="warm")
        nc.vector.memset(warm[:], 0.0)
        pw = psumw.tile([128, 64], F32, tag="pw")
        for _ in range(N_WARM_MM):
            nc.tensor.matmul(pw[:], warm[:, :128], warm[:, 128:], start=True, stop=True)
        dummy_act = const.tile([1, 8], F32, tag="dact")
        nc.scalar.activation(dummy_act[:], warm[:1, :8], AF.Exp)

        for gi in (1, 3):
            if gi < len(groups):
                wg_dma(nc.scalar, gi)

        # ---- front-loaded dist/exp (needs only T + S) ----
        kerns = {}

        def emit_distexp(k):
            dist = tmp.tile([LP, LJ, BLOC, D], F32, tag="dist")
            nc.scalar.activation(
                dist[:], Tt[:], AF.Abs,
                bias=S_sb[:LP, K + k : K + k + 1], scale=1.0,
            )
            kern = const.tile([LP, LJ, BLOC, D], BF16, tag=f"kern{k}")
            nc.scalar.activation(kern[:], dist[:], AF.Exp, scale=S_sb[:LP, k : k + 1])
            kerns[k] = kern

        osb = const.tile([K, BLOC, D], F32)
        po = psum1.tile([K, BLOC, D], F32)  # L-sums, one bank, rows = k

        Sps, Qs = {}, {}

        def stage_product(i):
            k = order[i]
            if nonzero[k]:
                emit_distexp(k)
            w = wslot(i)
            Sp = tmp.tile([LP, NF, LJ, BLOC, D], BF16, tag="Sp")
            for f0 in (0, 2):
                wap = bass.AP(
                    tensor=w.tensor,
                    offset=w.offset + f0 * LJ * D,
                    ap=[w.ap[0], [LJ * D, 2], [D, LJ], [0, BLOC], [1, D]],
                )
                nc.vector.tensor_tensor(
                    Sp[:, f0 : f0 + 2], Dp[:, f0 : f0 + 2], wap, AX.mult
                )
            Sps[i] = Sp
            if nonzero[k]:
                Q = tmp.tile([LP, LJ, BLOC, D], BF16, tag="Q")
                nc.vector.tensor_tensor(Q[:], Sp[:, 3], kerns[k][:], AX.mult)
                Qs[i] = Q

        def stage_mms(i):
            Sp = Sps[i]
            qterm = Qs.get(i)
            w = wslot(i)
            # bias first: only needs W (starts accumulation before products)
            terms = [_bc(w[:, NF])]
            terms += [Sp[:, 0], Sp[:, 1], Sp[:, 2]]
            terms.append(qterm[:] if qterm is not None else Sp[:, 3])
            pre = psum.tile([LP, LJ, BLOC, D], F32, tag="pre")
            for ti, t in enumerate(terms):
                for j in range(LJ):
                    nc.tensor.matmul(
                        pre[:, j], EYE[:LP, :LP], t[:, j],
                        start=(ti == 0), stop=(ti == len(terms) - 1),
                    )
            return pre

        pres = {}

        def stage_back(i, first, last):
            k = order[i]
            pre = pres[i]
            w = wslot(i)
            lat = tmp.tile([LP, LJ, BLOC, D], BF16, tag="lat")
            nc.scalar.activation(lat[:], pre[:], AF.Relu)
            z = tmp.tile([LP, LJ, BLOC, D], BF16, tag="z")
            nc.vector.tensor_tensor(z[:], lat[:], _bc(w[:, NF + 1]), AX.mult)
            for j in range(LJ):
                nc.tensor.matmul(
                    po[:, :, :],
                    E_sb[:LP, k * K : (k + 1) * K],
                    z[:, j],
                    start=(first and j == 0),
                    stop=(last and j == LJ - 1),
                )

        stage_product(0)
        stage_product(1)
        for i in range(K):
            if i + 2 < K:
                stage_product(i + 2)
            pres[i] = stage_mms(i)
            if i >= 1:
                stage_back(i - 1, first=(i == 1), last=False)
        stage_back(K - 1, first=False, last=True)

        # ---- epilogue: out = relu(po + 200*bv) ----
        nc.vector.tensor_tensor(osb[:], po[:], _bc(BV_sb[:]), AX.add)
        nc.vector.tensor_scalar_max(osb[:], osb[:], 0.0)
        nc.scalar.dma_start(out=out_d[:].rearrange("b k d -> k b d"), in_=osb[:])

    nc.compile()
    return nc


_NC_CACHE = {}


def _get_nc(nonzero):
    key = tuple(nonzero)
    if key not in _NC_CACHE:
        _NC_CACHE[key] = build_bass(key)
    return _NC_CACHE[key]


def make_in_maps(X, T, M, DT, alpha, w_v, w_t, b_v, b_t):
    X = np.asarray(X, np.float32)
    T = np.asarray(T, np.float32)
    M = np.asarray(M, np.float32)
    DT = np.asarray(DT, np.float32)
    w_t = np.asarray(w_t, np.float32)
    w_v = np.asarray(w_v, np.float32)
    b_t = np.asarray(b_t, np.float32)
    b_v = np.asarray(b_v, np.float32)
    alpha = np.asarray(alpha, np.float32).reshape(K)

    nonzero = tuple(bool(a > 0) for a in alpha)
    order = k_order(nonzero)

    # weight pack: [K, L, 6, D] with f-order (wt0, wt1, wt3, wt2, 4bt, wv)
    W = np.empty((K, L, NF + 2, D), np.float32)
    W[:, :, 0] = w_t[:, :, :, 0]
    W[:, :, 1] = w_t[:, :, :, 1]
    W[:, :, 2] = w_t[:, :, :, 3]
    W[:, :, 3] = w_t[:, :, :, 2]
    W[:, :, 4] = 4.0 * b_t[:, :, :, 0]
    W[:, :, 5] = w_v
    # -> [K, LP, 6, LJ, D] with l = j*LP + p, k's in consumption order
    W = W.reshape(K, LJ, LP, NF + 2, D).transpose(0, 2, 3, 1, 4)[list(order)]
    W = np.ascontiguousarray(W).astype(NPBF)

    # CF: [128, 26+64] f32 = S | 200*b_v (padded to 128 rows)
    CF = np.zeros((128, 2 * K + D), np.float32)
    CF[:, :K] = -np.maximum(alpha.reshape(1, K), 0.0)
    CF[:, K : 2 * K] = -REF_TIME.reshape(1, K)
    CF[:K, 2 * K :] = float(L) * b_v[:, 0, :]
    # CB: [128, 100+169] bf16 = eye(100) | selector columns
    CB = np.zeros((128, LP + K * K), np.float32)
    CB[:LP, :LP] = np.eye(LP)
    for k in range(K):
        CB[:, LP + k * K + k] = 1.0
    CB = CB.astype(NPBF)

    def tr4(A):
        # [BLOC, L, D] -> [LJ, LP, BLOC, D]
        return np.ascontiguousarray(
            A.reshape(BLOC, LJ, LP, D).transpose(1, 2, 0, 3)
        )

    in_maps = []
    for c in range(NCORES):
        b0 = c * BLOC
        bs = slice(b0, b0 + BLOC)
        D4 = np.stack(
            [
                tr4(X[bs].astype(NPBF)),
                tr4(DT[bs].astype(NPBF)),
                tr4(M[bs].astype(NPBF)),
                tr4(np.maximum(X[bs], 0.0).astype(NPBF)),
            ]
        )
        in_maps.append(
            {
                "D4": D4,
                "T4": tr4(T[bs]),
                "W": W,
                "CF": CF,
                "CB": CB,
            }
        )
    return in_maps, nonzero


def kernel(X, T, M, DT, alpha, w_v, w_t, b_v, b_t):
    in_maps, nonzero = make_in_maps(X, T, M, DT, alpha, w_v, w_t, b_v, b_t)
    nc = _get_nc(nonzero)
    res = run_bass_kernel_spmd(nc, in_maps, core_ids=list(range(NCORES)))
    out = np.concatenate([res.results[c]["out"] for c in range(NCORES)], axis=0)
    return out.astype(np.float32)
